# revision 1
# baseline (speedup 1.0000x reference)
"""Multi-head attention (B=2, S=2048, D=768, H=12) on 8 TRN2 NeuronCores.

Sharding: core c -> batch b = c//4, head-group g = c%4 (3 heads of 64 each).
Each core computes q/k/v projections for its 3 heads, masked softmax
attention (transposed-energy formulation, denominator via a ones column
appended to v), and a partial output projection against its 192 columns of
Wo. Host sums the 4 partial outputs per batch element.

Device layout notes:
  - Projections produce qT/kT/vT (d on partitions) via W-stationary matmuls.
  - E^T[k,q] accumulates with K=64; heads 0/1 are packed in partitions
    0-63/64-127 of one tile so their matmuls run on distinct PE row groups.
  - P^T = exp(scale*E^T) * maskT(0/1, bf16) ; out_un^T = [v|1]^T-style
    matmul (ones column -> denominator row 64 of the PSUM tile).
  - Normalisation happens on the small out_un^T (65 x 512) tiles, not on P.
  - All f32 matmuls are issued as float32r (full-rate for free dim >= 256).
"""

import os
import sys

sys.path.insert(0, "/opt/trn_rl_repo")

from contextlib import ExitStack

import ml_dtypes
import numpy as np

import concourse.bass as bass
import concourse.mybir as mybir
import concourse.tile as tile
from concourse import bacc
from concourse.bass import ds
from concourse.bass_utils import run_bass_kernel_spmd
from concourse.masks import make_identity

F32 = mybir.dt.float32
F32R = mybir.dt.float32r
BF16 = mybir.dt.bfloat16

SEQ = 2048
D = 768
HD = 64
GD = 192          # head-group width = 3 heads * 64
QB = 512          # q-block (free dim of E^T matmuls)
NQB = SEQ // QB   # 4
KT = SEQ // 128   # 16 k-tiles
SCALE = float(1.0 / np.sqrt(np.float32(D)))

_CACHE = {}


def _install_profile_hook():
    """The image's antenv lacks axon_hooks; synthesize it so
    run_bass_kernel_spmd(trace=True) can reach the NTFF profiler in
    libaxon_pjrt.so (same ctypes shim trn_agent_boot uses)."""
    import types

    if "antenv.axon_hooks" in sys.modules:
        return
    sys.path.insert(0, "/root/.axon_site")
    try:
        from trn_agent_boot.trn_boot import _ntff_profile_via_ctypes
        hook = _ntff_profile_via_ctypes("/opt/axon/libaxon_pjrt.so")
    except Exception:
        hook = None
    import concourse.bass_utils as _bu

    _bu.upload_artifacts = lambda tmpdir: tmpdir  # no artifact bucket here
    mod = types.ModuleType("antenv.axon_hooks")
    mod.get_axon_ntff_profile_hook = lambda: hook
    mod.set_axon_ntff_profile_hook = lambda h: None
    sys.modules["antenv.axon_hooks"] = mod



def _build():
    nc = bacc.Bacc(None)

    xqT = nc.declare_dram_parameter("xqT", [D, SEQ], BF16, isOutput=False)
    xkT = nc.declare_dram_parameter("xkT", [D, SEQ], BF16, isOutput=False)
    xvT = nc.declare_dram_parameter("xvT", [D, SEQ], BF16, isOutput=False)
    wqT = nc.declare_dram_parameter("wqT", [D, GD], BF16, isOutput=False)
    wkT = nc.declare_dram_parameter("wkT", [D, GD], BF16, isOutput=False)
    wvT = nc.declare_dram_parameter("wvT", [D, GD], BF16, isOutput=False)
    woT = nc.declare_dram_parameter("woT", [GD, D], BF16, isOutput=False)
    maskT = nc.declare_dram_parameter("maskT", [SEQ, SEQ], BF16, isOutput=False)
    out = nc.declare_dram_parameter("out", [SEQ, D], F32, isOutput=True)

    with tile.TileContext(nc) as tc, ExitStack() as ctx:
        Exp = mybir.ActivationFunctionType.Exp

        cpool = ctx.enter_context(tc.tile_pool(name="const", bufs=1))
        ident = cpool.tile([128, 128], BF16)
        make_identity(nc, ident[:])

        # ---- persistent activations --------------------------------------
        pp = ctx.enter_context(tc.tile_pool(name="persist", bufs=1))
        qA = pp.tile([128, SEQ], BF16, tag="qA")   # heads 0 (p0-63) and 1 (p64-127)
        qB = pp.tile([64, SEQ], BF16, tag="qB")    # head 2
        kA = pp.tile([128, SEQ], BF16, tag="kA")
        kB = pp.tile([64, SEQ], BF16, tag="kB")
        vaug = [pp.tile([128, KT, HD + 1], BF16, tag=f"vaug{h}", name=f"vaug{h}") for h in range(3)]
        onorm = [pp.tile([64, SEQ], BF16, tag=f"onorm{h}", name=f"onorm{h}") for h in range(3)]
        wo_sb = [pp.tile([64, D], BF16, tag=f"wo{h}", name=f"wo{h}") for h in range(3)]

        for h in range(3):
            nc.sync.dma_start(wo_sb[h][:], woT[ds(h * 64, 64), :])
            # ones column for the softmax denominator
            nc.vector.memset(vaug[h][:, :, HD : HD + 1], 1.0)

        # ---- phase 1: projections + v transpose --------------------------
        wp = ctx.enter_context(tc.tile_pool(name="wp", bufs=1))
        xp = ctx.enter_context(tc.tile_pool(name="xp", bufs=9))
        vtp = ctx.enter_context(tc.tile_pool(name="vt", bufs=1))
        with tc.tile_pool(name="pj_ps", bufs=2, space="PSUM") as pj_ps, \
             tc.tile_pool(name="tr_ps", bufs=2, space="PSUM") as tr_ps:

            w_sb = {}
            for name, wT in (("q", wqT), ("k", wkT), ("v", wvT)):
                w_sb[name] = wp.tile([128, 6, GD], BF16, tag=f"w{name}", name=f"w_{name}")
                nc.sync.dma_start(
                    w_sb[name][:], wT.rearrange("(ko ki) d -> ki ko d", ki=128)
                )

            vtA = vtp.tile([128, SEQ], BF16, tag="vtA")
            vtB = vtp.tile([64, SEQ], BF16, tag="vtB")

            dests = {"q": (qA, qB), "k": (kA, kB), "v": (vtA, vtB)}
            for name, xT in (("q", xqT), ("k", xkT), ("v", xvT)):
                for nb2 in range(2):  # halve DMA count: 1024-wide x tiles
                    xk = []
                    for k in range(6):
                        xt = xp.tile(
                            [128, 1024], BF16, tag="x", name=f"x_{name}_{nb2}_{k}"
                        )
                        nc.gpsimd.dma_start(
                            xt[:], xT[ds(k * 128, 128), ds(nb2 * 1024, 1024)]
                        )
                        xk.append(xt)
                    for half in range(2):
                        n = nb2 * 2 + half
                        for mt in range(2):
                            mw = 128 if mt == 0 else 64
                            ps = pj_ps.tile([128, QB], F32, tag="pjps")
                            for k in range(6):
                                nc.tensor.matmul(
                                    ps[0:mw, :],
                                    lhsT=w_sb[name][:, k, ds(mt * 128, mw)],
                                    rhs=xk[k][:, ds(half * QB, QB)],
                                    start=(k == 0),
                                    stop=(k == 5),
                                )
                            dst = dests[name][0] if mt == 0 else dests[name][1]
                            if name == "v":
                                nc.vector.tensor_copy(
                                    dst[0:mw, ds(n * QB, QB)], ps[0:mw, :]
                                )
                            else:
                                nc.scalar.copy(
                                    dst[0:mw, ds(n * QB, QB)], ps[0:mw, :]
                                )

            # transpose vT -> v_aug (bf16), per 128-seq block
            for s in range(KT):
                ptA = tr_ps.tile([128, 128], BF16, tag="ptA")
                nc.tensor.transpose(ptA[:], vtA[:, ds(s * 128, 128)], ident[:])
                nc.vector.tensor_copy(vaug[0][:, s, 0:HD], ptA[:, 0:64])
                nc.vector.tensor_copy(vaug[1][:, s, 0:HD], ptA[:, 64:128])
                ptB = tr_ps.tile([128, 64], BF16, tag="ptB")
                nc.tensor.transpose(
                    ptB[:], vtB[0:64, ds(s * 128, 128)], ident[0:64, 0:64]
                )
                nc.vector.tensor_copy(vaug[2][:, s, 0:HD], ptB[:, 0:64])

        # ---- phase 2: attention ------------------------------------------
        mp = ctx.enter_context(tc.tile_pool(name="mp", bufs=2))
        pp2 = ctx.enter_context(tc.tile_pool(name="pp2", bufs=3))
        rp = ctx.enter_context(tc.tile_pool(name="rp", bufs=2))
        with tc.tile_pool(name="e_ps", bufs=2, space="PSUM") as e_ps, \
             tc.tile_pool(name="ou_ps", bufs=2, space="PSUM") as ou_ps:

            q_of = (qA, qA, qB)
            k_of = (kA, kA, kB)
            pbase = (0, 64, 0)

            for n in range(NQB):
                mask_t = mp.tile([128, KT, QB], BF16, tag="mask")
                for j in range(KT):
                    nc.gpsimd.dma_start(
                        mask_t[:, j, :],
                        maskT[ds(j * 128, 128), ds(n * QB, QB)],
                    )
                P = [pp2.tile([128, KT, QB], BF16, tag="P", name=f"P{n}_{i}") for i in range(3)]
                for grp in range(KT // 2):
                    for h in range(3):
                        e = e_ps.tile([128, 2, QB], F32, tag="e")
                        for mm in range(2):
                            m = grp * 2 + mm
                            p0 = pbase[h]
                            nc.tensor.matmul(
                                e[:, mm, :],
                                lhsT=k_of[h][
                                    p0 : p0 + 64, ds(m * 128, 128)
                                ],
                                rhs=q_of[h][p0 : p0 + 64, ds(n * QB, QB)],
                                start=True,
                                stop=True,
                            )
                        sl = ds(grp * 2, 2)
                        nc.scalar.activation(
                            P[h][:, sl, :], e[:, :, :], Exp, scale=SCALE
                        )
                        nc.vector.tensor_mul(
                            P[h][:, sl, :], P[h][:, sl, :], mask_t[:, sl, :]
                        )
                for h in range(3):
                    ou = ou_ps.tile([HD + 1, QB], F32, tag="ou")
                    for m in range(KT):
                        nc.tensor.matmul(
                            ou[:],
                            lhsT=vaug[h][:, m, :],
                            rhs=P[h][:, m, :],
                            start=(m == 0),
                            stop=(m == KT - 1),
                        )
                    r1 = rp.tile([1, QB], F32, tag="r1")
                    nc.vector.reciprocal(r1[:], ou[HD : HD + 1, :])
                    rb = rp.tile([64, QB], F32, tag="rb")
                    nc.gpsimd.partition_broadcast(rb[:], r1[:])
                    nc.vector.tensor_mul(
                        onorm[h][:, ds(n * QB, QB)], ou[0:HD, :], rb[:]
                    )

        # ---- phase 3: output projection ----------------------------------
        op = ctx.enter_context(tc.tile_pool(name="op", bufs=3))
        with tc.tile_pool(name="fp", bufs=2, space="PSUM") as f_ps:
            for mq in range(SEQ // 128):
                fp = f_ps.tile([128, D], F32, tag="f")
                for n0, nw in ((0, 512), (512, 256)):
                    for h in range(3):
                        nc.tensor.matmul(
                            fp[:, ds(n0, nw)],
                            lhsT=onorm[h][:, ds(mq * 128, 128)],
                            rhs=wo_sb[h][:, ds(n0, nw)],
                            start=(h == 0),
                            stop=(h == 2),
                        )
                o_sb = op.tile([128, D], F32, tag="o")
                nc.scalar.copy(o_sb[:], fp[:])
                nc.gpsimd.dma_start(out[ds(mq * 128, 128), :], o_sb[:])

    nc.compile()
    return nc


def kernel(Q, K, V, mask, Wq, Wk, Wv, Wo):
    if "nc" not in _CACHE:
        _CACHE["nc"] = _build()
    nc = _CACHE["nc"]

    maskT_bf = np.ascontiguousarray(
        (mask[0, 0].T != 0).astype(ml_dtypes.bfloat16)
    )
    in_maps = []
    for c in range(8):
        b, g = c // 4, c % 4
        sl = slice(g * GD, (g + 1) * GD)
        in_maps.append(
            {
                "xqT": np.ascontiguousarray(Q[b].T.astype(ml_dtypes.bfloat16)),
                "xkT": np.ascontiguousarray(K[b].T.astype(ml_dtypes.bfloat16)),
                "xvT": np.ascontiguousarray(V[b].T.astype(ml_dtypes.bfloat16)),
                "wqT": np.ascontiguousarray(Wq[sl, :].T.astype(ml_dtypes.bfloat16)),
                "wkT": np.ascontiguousarray(Wk[sl, :].T.astype(ml_dtypes.bfloat16)),
                "wvT": np.ascontiguousarray(Wv[sl, :].T.astype(ml_dtypes.bfloat16)),
                "woT": np.ascontiguousarray(Wo[:, sl].T.astype(ml_dtypes.bfloat16)),
                "maskT": maskT_bf,
            }
        )

    _install_profile_hook()
    res = run_bass_kernel_spmd(
        nc,
        in_maps,
        core_ids=list(range(8)),
        trace=bool(int(os.environ.get("KERNEL_PROFILE", "0"))),
    )
    _CACHE["last_exec_ns"] = res.exec_time_ns

    out = np.zeros((2, SEQ, D), dtype=np.float32)
    for c in range(8):
        out[c // 4] += res.results[c]["out"]
    return out



# revision 15
# speedup vs baseline: 1.0942x; 1.0942x over previous
"""Multi-head attention (B=2, S=2048, D=768, H=12) on 8 TRN2 NeuronCores.

Sharding: core c -> batch b = c//4, head-group g = c%4 (3 heads of 64 each).

v2 design (ACT-bound pipeline):
  - Host pre-packs all DRAM tensors so every DMA is contiguous 2KB+ lines.
  - Phase 1a: W-stationary q/k projections. Heads 0/1 land in qA/kA at
    partitions 0-63/64-127; head 2 lands in qB/kB and is duplicated into
    both partition halves (SBUF->SBUF DMA) for row-balanced E matmuls.
  - Per 512-wide q-block n: E^T = K^T Q per head with K=64 matmuls placed
    on alternating PE row halves; exp on ACT (the kernel bottleneck:
    96 x ~1.04us); mask multiply on DVE; aug out-matmul [v|1]^T P^T
    (denominator = PSUM row 64); reciprocal_approx_fast + gpsimd broadcast
    + DVE mul for normalisation; per-n Wo projection and f32 store.
  - v projection is X-stationary (v tiles land [seq,192] directly -> no PE
    transposes) and is interleaved under the n=0 attention block.
  - ACT runs exp only; all PSUM->SBUF copies are on DVE/gpsimd.
"""

import os
import sys

sys.path.insert(0, "/opt/trn_rl_repo")

from contextlib import ExitStack

import ml_dtypes
import numpy as np

import concourse.bass as bass
import concourse.mybir as mybir
import concourse.tile as tile
from concourse import bacc
from concourse.bass import ds
from concourse.bass_utils import run_bass_kernel_spmd

F32 = mybir.dt.float32
BF16 = mybir.dt.bfloat16

SEQ = 2048
D = 768
HD = 64
GD = 192          # head-group width = 3 heads * 64
QB = 512          # q-block (free dim of E^T matmuls)
NQB = SEQ // QB   # 4
KT = SEQ // 128   # 16 k-tiles
KC = D // 128     # 6 contraction chunks
SCALE = float(1.0 / np.sqrt(np.float32(D)))

_CACHE = {}


def _install_profile_hook():
    """The image's antenv lacks axon_hooks; synthesize it so
    run_bass_kernel_spmd(trace=True) can reach the NTFF profiler in
    libaxon_pjrt.so (same ctypes shim trn_agent_boot uses)."""
    import types

    if "antenv.axon_hooks" in sys.modules:
        return
    sys.path.insert(0, "/root/.axon_site")
    try:
        from trn_agent_boot.trn_boot import _ntff_profile_via_ctypes
        hook = _ntff_profile_via_ctypes("/opt/axon/libaxon_pjrt.so")
    except Exception:
        hook = None
    import concourse.bass_utils as _bu

    _bu.upload_artifacts = lambda tmpdir: tmpdir  # no artifact bucket here
    mod = types.ModuleType("antenv.axon_hooks")
    mod.get_axon_ntff_profile_hook = lambda: hook
    mod.set_axon_ntff_profile_hook = lambda h: None
    sys.modules["antenv.axon_hooks"] = mod


def _build():
    nc = bacc.Bacc(None)
    Exp = mybir.ActivationFunctionType.Exp

    xq = nc.declare_dram_parameter("xq", [128, KC, SEQ], BF16, isOutput=False)
    xk = nc.declare_dram_parameter("xk", [128, KC, SEQ], BF16, isOutput=False)
    xv = nc.declare_dram_parameter("xv", [128, KC, SEQ], BF16, isOutput=False)
    wq = nc.declare_dram_parameter("wq", [128, KC, GD], BF16, isOutput=False)
    wk = nc.declare_dram_parameter("wk", [128, KC, GD], BF16, isOutput=False)
    wv = nc.declare_dram_parameter("wv", [128, KC, GD], BF16, isOutput=False)
    woT = nc.declare_dram_parameter("woT", [GD, D], BF16, isOutput=False)
    maskP = nc.declare_dram_parameter("maskP", [128, NQB, KT, QB], BF16,
                                      isOutput=False)
    out = nc.declare_dram_parameter("out", [SEQ, D], F32, isOutput=True)

    with tile.TileContext(nc) as tc, ExitStack() as ctx:
        # ---- persistent SBUF ------------------------------------------------
        pp = ctx.enter_context(tc.tile_pool(name="persist", bufs=1))
        qA = pp.tile([128, SEQ], BF16, tag="qA")    # heads 0/1 at p0-63/64-127
        kA = pp.tile([128, SEQ], BF16, tag="kA")
        qB = pp.tile([128, SEQ], BF16, tag="qB")    # head 2, both halves
        kB = pp.tile([128, SEQ], BF16, tag="kB")
        vaug = pp.tile([128, KT, 3, HD + 1], BF16, tag="vaug")
        onorm = [pp.tile([64, SEQ], BF16, tag=f"onorm{h}",
                         name=f"onorm{h}") for h in range(3)]
        wo_sb = [pp.tile([64, D], BF16, tag=f"wo{h}", name=f"wo{h}")
                 for h in range(3)]
        w_sb = {}
        for name, wT in (("q", wq), ("k", wk), ("v", wv)):
            w_sb[name] = pp.tile([128, KC, GD], BF16, tag=f"w{name}",
                                 name=f"w_{name}")
            nc.sync.dma_start(w_sb[name][:], wT[:, :, :])
        for h in range(3):
            nc.sync.dma_start(wo_sb[h][:], woT[ds(h * 64, 64), :])
        nc.vector.memset(vaug[:, :, :, HD:HD + 1], 1.0)

        xp = ctx.enter_context(tc.tile_pool(name="xp", bufs=14))
        mp = ctx.enter_context(tc.tile_pool(name="mp", bufs=2))
        pp2 = ctx.enter_context(tc.tile_pool(name="pp2", bufs=4))
        rp = ctx.enter_context(tc.tile_pool(name="rp", bufs=6))
        op = ctx.enter_context(tc.tile_pool(name="op", bufs=3))

        # ==== phase 1a: q/k projections (W-stationary) ======================
        with tc.tile_pool(name="pj_ps", bufs=3, space="PSUM") as pj_ps:
            for nb2 in range(2):
                xt = {}
                for name, xT in (("q", xq), ("k", xk)):
                    xt[name] = []
                    for k in range(KC):
                        t = xp.tile([128, 1024], BF16, tag="x",
                                    name=f"x_{name}_{nb2}_{k}")
                        eng = nc.sync if (k % 2 == 0) else nc.scalar
                        eng.dma_start(t[:], xT[:, k, ds(nb2 * 1024, 1024)])
                        xt[name].append(t)
                for half in range(2):
                    n = nb2 * 2 + half
                    sl = ds(n * QB, QB)
                    hsl = ds(half * QB, QB)
                    ps = {}
                    for name in ("q", "k"):
                        ps[name] = pj_ps.tile([128, QB], F32, tag="pjps",
                                              name=f"pj_{name}_{n}")
                        for k in range(KC):
                            nc.tensor.matmul(
                                ps[name][:],
                                lhsT=w_sb[name][:, k, 0:128],
                                rhs=xt[name][k][:, hsl],
                                start=(k == 0), stop=(k == KC - 1),
                            )
                    # head 2 (M=64): q -> psum[0:64], k -> psum[64:128]
                    psB = pj_ps.tile([128, QB], F32, tag="pjps",
                                     name=f"pj_B_{n}")
                    for k in range(KC):
                        nc.tensor.matmul(
                            psB[0:64, :],
                            lhsT=w_sb["q"][:, k, 128:192],
                            rhs=xt["q"][k][:, hsl],
                            start=(k == 0), stop=(k == KC - 1),
                        )
                    for k in range(KC):
                        nc.tensor.matmul(
                            psB[64:128, :],
                            lhsT=w_sb["k"][:, k, 128:192],
                            rhs=xt["k"][k][:, hsl],
                            start=(k == 0), stop=(k == KC - 1),
                        )
                    nc.vector.tensor_copy(qA[:, sl], ps["q"][:])
                    nc.vector.tensor_copy(kA[:, sl], ps["k"][:])
                    nc.vector.tensor_copy(qB[0:64, sl], psB[0:64, :])
                    nc.vector.tensor_copy(kB[64:128, sl], psB[64:128, :])
                    # duplicate head-2 halves (cross-partition -> DMA)
                    nc.sync.dma_start(qB[64:128, sl], qB[0:64, sl])
                    nc.sync.dma_start(kB[0:64, sl], kB[64:128, sl])

        # ==== phase 2 (+v proj under n=0, +per-n Wo projection) =============
        # PSUM stack: e_ps(4 banks) + ou_ps(3) + v_ps(1, closed after n=0,
        # replaced by fp_ps(1)) = 8 banks
        with tc.tile_pool(name="e_ps", bufs=2, space="PSUM") as e_ps, \
             tc.tile_pool(name="ou_ps", bufs=3, space="PSUM") as ou_ps:
            v_stack = ExitStack()
            v_ps = v_stack.enter_context(
                tc.tile_pool(name="vps", bufs=1, space="PSUM"))
            fp_holder = {}

            xvt = []
            for k in range(KC):
                halves = []
                for hh in range(2):
                    t = xp.tile([128, 1024], BF16, tag="x",
                                name=f"x_v_{k}_{hh}")
                    eng = (nc.sync, nc.gpsimd)[(2 * k + hh) % 2]
                    eng.dma_start(t[:], xv[:, k, ds(hh * 1024, 1024)])
                    halves.append(t)
                xvt.append(halves)

            def emit_v_tile(s):
                vp = v_ps.tile([128, 3, HD], F32, tag="vps")
                for k in range(KC):
                    nc.tensor.matmul(
                        vp[:, :, :],
                        lhsT=xvt[k][s // 8][:, ds((s % 8) * 128, 128)],
                        rhs=w_sb["v"][:, k, :],
                        start=(k == 0), stop=(k == KC - 1),
                    )
                nc.vector.tensor_copy(vaug[:, s, :, 0:HD], vp[:, :, :])

            def emit_wo(mq):
                """Wo projection for one 128-row q tile (needs onorm of
                n-block mq//4). Single tile_position per accum group."""
                msl = ds(mq * 128, 128)
                o_sb = op.tile([128, D], F32, tag="o", name=f"o_{mq}")
                for n0, nw in ((0, 512), (512, 256)):
                    fp = fp_holder["pool"].tile([128, nw], F32, tag="fp",
                                                name=f"fp_{mq}_{n0}")
                    for h in range(3):
                        nc.tensor.matmul(fp[:], lhsT=onorm[h][:, msl],
                                         rhs=wo_sb[h][:, ds(n0, nw)],
                                         start=(h == 0), stop=(h == 2))
                    nc.vector.tensor_copy(o_sb[:, ds(n0, nw)], fp[:])
                nc.gpsimd.dma_start(out[msl, :], o_sb[:])

            for n in range(NQB):
                nsl = ds(n * QB, QB)
                mask_t = mp.tile([128, KT, QB], BF16, tag="mask")
                for c in range(4):
                    eng = (nc.sync, nc.gpsimd, nc.sync, nc.gpsimd)[c]
                    eng.dma_start(mask_t[:, ds(c * 4, 4), :],
                                  maskP[:, n, ds(c * 4, 4), :])
                P = [pp2.tile([128, KT, QB], BF16, tag="P",
                              name=f"P{n}_{i}") for i in range(3)]

                for grp in range(KT // 2):
                    m0, m1 = grp * 2, grp * 2 + 1
                    e = [e_ps.tile([128, 2, QB], F32, tag="e",
                                   name=f"e{n}_{grp}_{h}") for h in range(3)]
                    # 3 row-balanced slots: [h0|h1](m0), [h2(m0)|h2(m1)],
                    # [h0|h1](m1)
                    nc.tensor.matmul(e[0][:, 0, :],
                                     lhsT=kA[0:64, ds(m0 * 128, 128)],
                                     rhs=qA[0:64, nsl], start=True, stop=True)
                    nc.tensor.matmul(e[1][:, 0, :],
                                     lhsT=kA[64:128, ds(m0 * 128, 128)],
                                     rhs=qA[64:128, nsl], start=True, stop=True)
                    nc.tensor.matmul(e[2][:, 0, :],
                                     lhsT=kB[0:64, ds(m0 * 128, 128)],
                                     rhs=qB[0:64, nsl], start=True, stop=True)
                    nc.tensor.matmul(e[2][:, 1, :],
                                     lhsT=kB[64:128, ds(m1 * 128, 128)],
                                     rhs=qB[64:128, nsl], start=True, stop=True)
                    nc.tensor.matmul(e[0][:, 1, :],
                                     lhsT=kA[0:64, ds(m1 * 128, 128)],
                                     rhs=qA[0:64, nsl], start=True, stop=True)
                    nc.tensor.matmul(e[1][:, 1, :],
                                     lhsT=kA[64:128, ds(m1 * 128, 128)],
                                     rhs=qA[64:128, nsl], start=True, stop=True)
                    psl = ds(grp * 2, 2)
                    for h in range(3):
                        nc.scalar.activation(P[h][:, psl, :], e[h][:, :, :],
                                             Exp, scale=SCALE)
                        nc.vector.tensor_mul(P[h][:, psl, :], P[h][:, psl, :],
                                             mask_t[:, psl, :])
                    if n == 0:
                        emit_v_tile(m0)
                        emit_v_tile(m1)

                # aug out-matmuls: ou[0:64] = out^T, ou[64] = denominator
                for h in range(3):
                    ou = ou_ps.tile([HD + 1, QB], F32, tag="ou",
                                    name=f"ou{n}_{h}")
                    for m in range(KT):
                        nc.tensor.matmul(
                            ou[:],
                            lhsT=vaug[:, m, h, :],
                            rhs=P[h][:, m, :],
                            start=(m == 0), stop=(m == KT - 1),
                        )
                    r1 = rp.tile([1, QB], F32, tag="r1")
                    nc.vector.reciprocal(r1[:], ou[HD:HD + 1, :])
                    rb = rp.tile([64, QB], F32, tag="rb")
                    nc.gpsimd.partition_broadcast(rb[:], r1[:])
                    nc.vector.tensor_mul(onorm[h][:, nsl],
                                             ou[0:HD, :], rb[:])

                if n == 0:
                    # v projection is done; swap its PSUM bank for fp
                    v_stack.close()
                    fp_holder["ctx"] = tc.tile_pool(name="fp_ps", bufs=1,
                                                    space="PSUM")
                    fp_holder["pool"] = fp_holder["ctx"].__enter__()

                # Wo projection for this n-block
                for mq in range(4 * n, 4 * n + 4):
                    emit_wo(mq)

            fp_holder["ctx"].__exit__(None, None, None)

    nc.compile()
    return nc


def kernel(Q, K, V, mask, Wq, Wk, Wv, Wo):
    if "nc" not in _CACHE:
        _CACHE["nc"] = _build()
    nc = _CACHE["nc"]

    bf = ml_dtypes.bfloat16

    def pack_x(x):  # [SEQ, D] -> [128, KC, SEQ]
        return np.ascontiguousarray(
            x.T.reshape(KC, 128, SEQ).transpose(1, 0, 2).astype(bf))

    def pack_w(W, sl):  # rows sl of W -> [128, KC, GD]
        return np.ascontiguousarray(
            W[sl, :].T.reshape(KC, 128, GD).transpose(1, 0, 2).astype(bf))

    maskT = (mask[0, 0].T != 0).astype(np.float32)
    maskP = np.ascontiguousarray(
        maskT.reshape(KT, 128, NQB, QB).transpose(1, 2, 0, 3).astype(bf))

    in_maps = []
    for c in range(8):
        b, g = c // 4, c % 4
        sl = slice(g * GD, (g + 1) * GD)
        woT = np.ascontiguousarray(Wo[:, sl].T.astype(bf))  # [192, 768]
        in_maps.append(
            {
                "xq": pack_x(np.asarray(Q[b])),
                "xk": pack_x(np.asarray(K[b])),
                "xv": pack_x(np.asarray(V[b])),
                "wq": pack_w(np.asarray(Wq), sl),
                "wk": pack_w(np.asarray(Wk), sl),
                "wv": pack_w(np.asarray(Wv), sl),
                "woT": woT,
                "maskP": maskP,
            }
        )

    _install_profile_hook()
    res = run_bass_kernel_spmd(
        nc,
        in_maps,
        core_ids=list(range(8)),
        trace=bool(int(os.environ.get("KERNEL_PROFILE", "0"))),
    )
    _CACHE["last_exec_ns"] = res.exec_time_ns

    out = np.zeros((2, SEQ, D), dtype=np.float32)
    for c in range(8):
        out[c // 4] += res.results[c]["out"]
    return out
